# revision 27
# baseline (speedup 1.0000x reference)
"""Trainium2 Bass kernel for fused Llama attention (nn_LlamaAttentionFused).

Reference computation (B=2, S=1024, H=4096, 32 Q heads, 8 KV heads, D=128):
    xq = x @ wq; xk = x @ wk; xv = x @ wv
    rope(xq, xk); causal GQA flash attention; out = attn @ wo

Sharding: 8-way tensor parallel over heads. Core c owns Q heads 4c..4c+3 and
KV head c (GQA groups stay together). Each core computes a full-shape partial
output (its heads' contribution through wo); the host sums the 8 partials.

v3 design notes:
  - Projections in bf16 (x, wq, wkv) with fp32 PSUM accumulation. q/k stored
    bf16; RoPE in bf16 on DVE (2x mode) with the sign of the sin term folded
    into the constant (3 tensor ops per head), overlapped with the next
    batch's projection matmuls. wo loads are emitted after batch 0's
    projections so they don't delay the first matmuls.
  - Attention in transposed layout: scoresT[k, q] = kT_blk.T @ qT as a single
    N<=1024 bf16 matmul per k-chunk into a bf16 PSUM bank; exp with no max
    subtraction (scores bounded); causal mask as post-exp 0/1 multiply on the
    diagonal block. PV uses the probs block as stationary and v-natural with
    a ones column as moving, so the softmax denominator lands per-partition
    in the PV psum; normalize via tensor_scalar; one PE transpose per qb.
  - Attention and output projection are fused per batch at q-block
    granularity: after PV of q-block qb for all 4 heads, the out-proj for
    that token block runs immediately, keeping the PE dense while ScalarE
    computes the next exps.

Device-side layouts (per core):
    xT   [4096, 2048] bf16  x transposed on host (tokens = 2 batches x 1024)
    wq   [4096, 512]  bf16
    wkv  [4096, 256]  bf16  wk|wv column-concat
    wo   [512, 4096]  bf16
    cosf [128, 1024]  bf16  freqs_cos.T stacked twice on partitions
    sinf [128, 1024]  bf16  [-freqs_sin.T ; +freqs_sin.T]
    out  [2048, 4096] bf16  partial output
"""

import numpy as np
import ml_dtypes

import concourse.bass as bass
import concourse.mybir as mybir
import concourse.tile as tile
from concourse import bacc
from concourse.bass_utils import run_bass_kernel_spmd
from concourse.masks import make_identity

F32 = mybir.dt.float32
F32R = mybir.dt.float32r
BF16 = mybir.dt.bfloat16

B = 2
S = 1024          # tokens per batch
H = 4096          # model dim
D = 128           # head dim
HQ = 4            # q heads per core
NT = B * S        # total tokens
SCALE = 1.0 / float(np.sqrt(D))

QB = S // 128     # 8 q-blocks of 128 per batch
KC = S // 128     # 8 k-chunks of 128 per batch
HC = H // 128     # 32 contraction chunks for the projections
VW = 132          # vnat row width: 128 v dims + ones col + 3 zero pad


def build_program():
    nc = bacc.Bacc("TRN2", target_bir_lowering=False, debug=False, num_devices=8)

    xT = nc.dram_tensor("xT", [H, NT], BF16, kind="ExternalInput").ap()
    # weights arrive pre-rearranged to partition-major SBUF layout so each
    # load is one descriptor per partition (128 vs 4096 -> ~30x faster issue)
    wqkv = nc.dram_tensor("wqkv", [128, HC * 6 * D], BF16,
                          kind="ExternalInput").ap()
    wo = nc.dram_tensor("wo", [128, HQ * H], BF16, kind="ExternalInput").ap()
    cosf = nc.dram_tensor("cosf", [128, S], BF16, kind="ExternalInput").ap()
    sinf = nc.dram_tensor("sinf", [128, S], BF16, kind="ExternalInput").ap()
    out = nc.dram_tensor("out", [NT, H], BF16, kind="ExternalOutput").ap()

    wqkv_r = wqkv.rearrange("p (n f) -> p n f", n=HC)  # [128, 32, 768]
    wo_r = wo.rearrange("p (n f) -> p n f", n=HQ)      # [128, 4, 4096]

    with tile.TileContext(nc) as tc:
        with (
            tc.tile_pool(name="const", bufs=1) as const,
            tc.tile_pool(name="weights", bufs=1) as weights,
            tc.tile_pool(name="stream", bufs=6) as stream,
            tc.tile_pool(name="acts", bufs=1) as acts,
            tc.tile_pool(name="work", bufs=2) as work,
            tc.tile_pool(name="stats", bufs=16) as stats,
        ):
            # ---- constants -------------------------------------------------
            ident = const.tile([128, 128], BF16)
            make_identity(nc, ident)

            # maskT01[k, q] = 1 where k <= q (valid causal in [k,q] layout)
            maskT01 = const.tile([128, 128], BF16)
            nc.gpsimd.memset(maskT01, 1.0)
            nc.gpsimd.affine_select(
                out=maskT01,
                in_=maskT01,
                compare_op=mybir.AluOpType.is_ge,
                fill=0.0,
                base=0,
                pattern=[[1, 128]],       # expr = -p + f >= 0 -> keep
                channel_multiplier=-1,
            )

            cosf_s = const.tile([128, S], BF16)
            sinf_s = const.tile([128, S], BF16)

            # ---- resident weights (wo loaded later, after proj b0) --------
            # single hc-ordered q|k|v weight stream: chunks arrive in exact
            # consumption order; tiny first chunk for a fast start
            wqkv_s = weights.tile([128, HC, 6 * D], BF16)
            bounds = [0, 1, 4] + list(range(8, HC + 1, 4))
            for a, e in zip(bounds, bounds[1:]):
                nc.scalar.dma_start(out=wqkv_s[:, a:e, :],
                                    in_=wqkv_r[:, a:e, :])
                if a == 4:
                    nc.scalar.dma_start(out=cosf_s, in_=cosf)
                    nc.scalar.dma_start(out=sinf_s, in_=sinf)
            wo_s = weights.tile([128, HQ, H], BF16)

            # ---- per-batch activations ------------------------------------
            qT = [acts.tile([128, HQ, S], BF16, tag=f"qT{b}", name=f"qT{b}")
                  for b in range(B)]
            kT = [acts.tile([128, S], BF16, tag=f"kT{b}", name=f"kT{b}")
                  for b in range(B)]
            vnat = [acts.tile([128, KC, VW], BF16, tag=f"vn{b}", name=f"vn{b}")
                    for b in range(B)]

            # ================= Phase 1: projections + rope =================
            with tc.tile_pool(name="pproj", bufs=1, space="PSUM") as psp:
                for b in range(B):
                    tok0 = b * S
                    for t in range(2):  # two 512-token chunks per batch
                        psq = [psp.tile([128, 512], F32, tag="pj", bufs=7,
                                        name=f"psq{b}{t}{d}") for d in range(HQ)]
                        psk = psp.tile([128, 512], F32, tag="pj", bufs=7)
                        psv = psp.tile([128, 512], F32, tag="pj", bufs=7)
                        for hc in range(HC):
                            xp = stream.tile([128, 512], BF16, tag="xp")
                            nc.sync.dma_start(
                                out=xp,
                                in_=xT[hc * 128:(hc + 1) * 128,
                                       tok0 + t * 512: tok0 + (t + 1) * 512],
                            )
                            first, last = hc == 0, hc == HC - 1
                            for d in range(HQ):
                                nc.tensor.matmul(
                                    psq[d],
                                    wqkv_s[:, hc, d * 128:(d + 1) * 128],
                                    xp,
                                    start=first, stop=last,
                                )
                            nc.tensor.matmul(psk, wqkv_s[:, hc, 512:640], xp,
                                             start=first, stop=last)
                            nc.tensor.matmul(psv, wqkv_s[:, hc, 640:768], xp,
                                             start=first, stop=last)
                        ts_ = slice(t * 512, (t + 1) * 512)
                        for d in range(HQ):
                            if d % 2 == 0:
                                nc.scalar.copy(qT[b][:, d, ts_], psq[d])
                            else:
                                nc.vector.tensor_copy(qT[b][:, d, ts_], psq[d])
                        nc.vector.tensor_copy(kT[b][:, ts_], psk)
                        # v: transpose to natural [tok, d] via PE
                        vT_sb = work.tile([128, 512], BF16, tag="vT", bufs=2)
                        nc.scalar.copy(vT_sb, psv)
                        ptr = psp.tile([128, 512], BF16, tag="trv", bufs=1,
                                       padded_shape=[128, 1024])
                        for i in range(4):
                            nc.tensor.transpose(
                                ptr[:, i * 128:(i + 1) * 128],
                                vT_sb[:, i * 128:(i + 1) * 128],
                                ident,
                            )
                        nc.vector.tensor_copy(
                            vnat[b][:, t * 4:(t + 1) * 4, 0:128],
                            ptr.rearrange("p (n f) -> p n f", n=4),
                        )
                    # ones column for the softmax denominator; zero pad
                    nc.gpsimd.memset(vnat[b][:, :, 128:129], 1.0)
                    nc.gpsimd.memset(vnat[b][:, :, 129:VW], 0.0)

                    if b == 0:
                        # wo loads ride behind the proj-b0 xp stream; needed
                        # only at the first fused out-proj, ~170us later.
                        for i in range(4):
                            nc.sync.dma_start(out=wo_s[:, i, :],
                                              in_=wo_r[:, i, :])

                    # rope on DVE (overlaps the next batch's projections)
                    def rope(dst):  # [128, S] bf16, in place
                        scr = work.tile([128, S], BF16, tag="scr", bufs=2)
                        nc.gpsimd.dma_start(out=scr[0:64, :], in_=dst[64:128, :])
                        nc.gpsimd.dma_start(out=scr[64:128, :], in_=dst[0:64, :])
                        nc.vector.tensor_mul(dst, dst, cosf_s)
                        nc.vector.tensor_mul(scr, scr, sinf_s)
                        nc.vector.tensor_add(dst, dst, scr)

                    for hh in range(HQ):
                        rope(qT[b][:, hh, :])
                    rope(kT[b])

            # ===== Phase 2: fused attention + output projection ============
            with tc.tile_pool(name="pattn", bufs=1, space="PSUM") as psa:
                aTs = {}

                def outproj(b, tb):
                    tok0 = b * S
                    aT = aTs.pop((b, tb))
                    for pair in range(4):
                        ev = work.tile([128, 1024], BF16, tag="ev", bufs=3)
                        for half in range(2):
                            ncol = pair * 2 + half
                            po = psa.tile([128, 512], F32, tag="op",
                                          bufs=2, name="psop")
                            for d in range(HQ):
                                nc.tensor.matmul(
                                    po,
                                    aT[:, d, :],
                                    wo_s[:, d,
                                         ncol * 512:(ncol + 1) * 512],
                                    start=(d == 0), stop=(d == HQ - 1),
                                )
                            dst = ev[:, half * 512:(half + 1) * 512]
                            if half == 0:
                                nc.scalar.copy(dst, po)
                            else:
                                nc.vector.tensor_copy(dst, po)
                        nc.sync.dma_start(
                            out=out[tok0 + tb * 128: tok0 + (tb + 1) * 128,
                                    pair * 1024:(pair + 1) * 1024],
                            in_=ev,
                        )

                for b in range(B):
                    PT = [work.tile([128, KC, S], BF16, tag=f"pt{hh}", bufs=1,
                                    name=f"PT{hh}") for hh in range(HQ)]
                    for kc in range(KC):
                        qlo = kc * 128
                        # QK^T transposed for all 4 heads at this k-chunk
                        spans = ([(qlo, 512), (512, S)] if qlo < 512
                                 else [(qlo, S)])
                        for hh in range(HQ):
                            for (a, e) in spans:
                                ps = psa.tile([128, 512], F32, tag="qk",
                                              bufs=3, name="psqk")
                                nc.tensor.matmul(
                                    ps[:, :e - a],
                                    kT[b][:, qlo:qlo + 128],
                                    qT[b][:, hh, a:e],
                                    start=True, stop=True,
                                )
                                nc.scalar.activation(
                                    PT[hh][:, kc, a:e],
                                    ps[:, :e - a],
                                    mybir.ActivationFunctionType.Exp,
                                    scale=SCALE,
                                )
                            nc.vector.tensor_mul(
                                PT[hh][:, kc, qlo:qlo + 128],
                                PT[hh][:, kc, qlo:qlo + 128],
                                maskT01,
                            )
                        # out-proj for the previous q-block rides here so
                        # the PE has work while ScalarE exps this k-chunk
                        if kc > 0:
                            outproj(b, kc - 1)
                        elif b > 0:
                            outproj(b - 1, KC - 1)
                        # PV for q-block qb=kc, all heads -> attnT block
                        qb = kc
                        aT = work.tile([128, HQ, 128], BF16, tag="aT", bufs=2)
                        aTs[(b, kc)] = aT
                        for hh in range(HQ):
                            pv = psa.tile([128, VW], F32, tag="pv", bufs=2,
                                          padded_shape=[128, 512], name="pspv")
                            for kc2 in range(qb + 1):
                                nc.tensor.matmul(
                                    pv,
                                    PT[hh][:, kc2, qb * 128:(qb + 1) * 128],
                                    vnat[b][:, kc2, :],
                                    start=(kc2 == 0), stop=(kc2 == qb),
                                )
                            rec = stats.tile([128, 1], F32, tag="st")
                            nc.vector.reciprocal(rec, pv[:, 128:129])
                            an = work.tile([128, 128], BF16, tag="an", bufs=2)
                            nc.vector.tensor_scalar_mul(an, pv[:, 0:128], rec)
                            ptr = psa.tile([128, 128], BF16, tag="tr2", bufs=1,
                                           padded_shape=[128, 1024], name="pstr")
                            nc.tensor.transpose(ptr, an, ident)
                            nc.vector.tensor_copy(aT[:, hh, :], ptr)
                outproj(B - 1, KC - 1)

    nc.compile()
    return nc


_NC = None


def _get_nc():
    global _NC
    if _NC is None:
        _NC = build_program()
    return _NC


def make_in_maps(x, wq, wk, wv, wo, freqs_cos, freqs_sin):
    bf = ml_dtypes.bfloat16
    x = np.asarray(x, np.float32)
    xT = np.ascontiguousarray(x.reshape(NT, H).T.astype(bf))
    cosT = np.asarray(freqs_cos, np.float32).T
    sinT = np.asarray(freqs_sin, np.float32).T
    cosf = np.ascontiguousarray(np.concatenate([cosT, cosT], 0).astype(bf))
    sinf = np.ascontiguousarray(np.concatenate([-sinT, sinT], 0).astype(bf))
    wq = np.asarray(wq, np.float32).astype(bf)
    wk = np.asarray(wk, np.float32).astype(bf)
    wv = np.asarray(wv, np.float32).astype(bf)
    wo = np.asarray(wo, np.float32).astype(bf)

    def pmajor(w):  # [H_in, F] -> [128, (H_in/128)*F] partition-major
        hin, f = w.shape
        return np.ascontiguousarray(
            w.reshape(hin // 128, 128, f).transpose(1, 0, 2).reshape(128, -1))

    in_maps = []
    for c in range(8):
        wq_pm = pmajor(np.ascontiguousarray(
            wq[:, c * 512:(c + 1) * 512])).reshape(128, 32, 512)
        wkv_pm = pmajor(np.ascontiguousarray(
            np.concatenate([wk[:, c * 128:(c + 1) * 128],
                            wv[:, c * 128:(c + 1) * 128]],
                           axis=1))).reshape(128, 32, 256)
        in_maps.append({
            "xT": xT,
            "wqkv": np.ascontiguousarray(
                np.concatenate([wq_pm, wkv_pm], axis=2).reshape(128, -1)),
            "wo": pmajor(np.ascontiguousarray(wo[c * 512:(c + 1) * 512, :])),
            "cosf": cosf,
            "sinf": sinf,
        })
    return in_maps


def kernel(x, wq, wk, wv, wo, freqs_cos, freqs_sin, start_pos=0, **_):
    nc = _get_nc()
    in_maps = make_in_maps(x, wq, wk, wv, wo, freqs_cos, freqs_sin)
    res = run_bass_kernel_spmd(nc, in_maps, list(range(8)))
    acc = res.results[0]["out"].astype(np.float32)
    for c in range(1, 8):
        acc = acc + res.results[c]["out"].astype(np.float32)
    return acc.reshape(B, S, H)


# revision 29
# speedup vs baseline: 1.0299x; 1.0299x over previous
"""Trainium2 Bass kernel for fused Llama attention (nn_LlamaAttentionFused).

Reference computation (B=2, S=1024, H=4096, 32 Q heads, 8 KV heads, D=128):
    xq = x @ wq; xk = x @ wk; xv = x @ wv
    rope(xq, xk); causal GQA flash attention; out = attn @ wo

Sharding: 8-way tensor parallel over heads. Core c owns Q heads 4c..4c+3 and
KV head c (GQA groups stay together). Each core computes a full-shape partial
output (its heads' contribution through wo); the host sums the 8 partials.

v3 design notes:
  - Projections in bf16 (x, wq, wkv) with fp32 PSUM accumulation. q/k stored
    bf16; RoPE in bf16 on DVE (2x mode) with the sign of the sin term folded
    into the constant (3 tensor ops per head), overlapped with the next
    batch's projection matmuls. wo loads are emitted after batch 0's
    projections so they don't delay the first matmuls.
  - Attention in transposed layout: scoresT[k, q] = kT_blk.T @ qT as a single
    N<=1024 bf16 matmul per k-chunk into a bf16 PSUM bank; exp with no max
    subtraction (scores bounded); causal mask as post-exp 0/1 multiply on the
    diagonal block. PV uses the probs block as stationary and v-natural with
    a ones column as moving, so the softmax denominator lands per-partition
    in the PV psum; normalize via tensor_scalar; one PE transpose per qb.
  - Attention and output projection are fused per batch at q-block
    granularity: after PV of q-block qb for all 4 heads, the out-proj for
    that token block runs immediately, keeping the PE dense while ScalarE
    computes the next exps.

Device-side layouts (per core):
    xT   [4096, 2048] bf16  x transposed on host (tokens = 2 batches x 1024)
    wq   [4096, 512]  bf16
    wkv  [4096, 256]  bf16  wk|wv column-concat
    wo   [512, 4096]  bf16
    cosf [128, 1024]  bf16  freqs_cos.T stacked twice on partitions
    sinf [128, 1024]  bf16  [-freqs_sin.T ; +freqs_sin.T]
    out  [2048, 4096] bf16  partial output
"""

import numpy as np
import ml_dtypes

import concourse.bass as bass
import concourse.mybir as mybir
import concourse.tile as tile
from concourse import bacc
from concourse.bass_utils import run_bass_kernel_spmd
from concourse.masks import make_identity

F32 = mybir.dt.float32
F32R = mybir.dt.float32r
BF16 = mybir.dt.bfloat16

B = 2
S = 1024          # tokens per batch
H = 4096          # model dim
D = 128           # head dim
HQ = 4            # q heads per core
NT = B * S        # total tokens
SCALE = 1.0 / float(np.sqrt(D))

QB = S // 128     # 8 q-blocks of 128 per batch
KC = S // 128     # 8 k-chunks of 128 per batch
HC = H // 128     # 32 contraction chunks for the projections
VW = 132          # vnat row width: 128 v dims + ones col + 3 zero pad


def build_program():
    nc = bacc.Bacc("TRN2", target_bir_lowering=False, debug=False, num_devices=8)

    xT = nc.dram_tensor("xT", [H, NT], BF16, kind="ExternalInput").ap()
    # weights arrive pre-rearranged to partition-major SBUF layout so each
    # load is one descriptor per partition (128 vs 4096 -> ~30x faster issue)
    wq = nc.dram_tensor("wq", [128, HC * HQ * D], BF16, kind="ExternalInput").ap()
    wkv = nc.dram_tensor("wkv", [128, HC * 2 * D], BF16, kind="ExternalInput").ap()
    wo = nc.dram_tensor("wo", [128, HQ * H], BF16, kind="ExternalInput").ap()
    cosf = nc.dram_tensor("cosf", [128, S], BF16, kind="ExternalInput").ap()
    sinf = nc.dram_tensor("sinf", [128, S], BF16, kind="ExternalInput").ap()
    out = nc.dram_tensor("out", [NT, H], BF16, kind="ExternalOutput").ap()

    wq_r = wq.rearrange("p (n f) -> p n f", n=HC)      # [128, 32, 512]
    wkv_r = wkv.rearrange("p (n f) -> p n f", n=HC)    # [128, 32, 256]
    wo_r = wo.rearrange("p (n f) -> p n f", n=HQ)      # [128, 4, 4096]

    with tile.TileContext(nc) as tc:
        with (
            tc.tile_pool(name="const", bufs=1) as const,
            tc.tile_pool(name="weights", bufs=1) as weights,
            tc.tile_pool(name="stream", bufs=6) as stream,
            tc.tile_pool(name="acts", bufs=1) as acts,
            tc.tile_pool(name="work", bufs=2) as work,
            tc.tile_pool(name="stats", bufs=16) as stats,
        ):
            # ---- PE warmup -------------------------------------------------
            # ~10 short throwaway matmuls during the initial DMA wait push the
            # HAM activity window so the first real matmuls run at 2.4GHz.
            # Sized to finish before the first projection matmul needs the
            # aliased PSUM bank (no write-after-read stall).
            warm = const.tile([128, 256], BF16, name="warm")
            nc.gpsimd.memset(warm, 1.0)
            with tc.tile_pool(name="pwarm", bufs=1, space="PSUM") as pw:
                pswarm = pw.tile([128, 256], F32, tag="w")
                for _ in range(10):
                    nc.tensor.matmul(pswarm, warm[:, 0:128], warm,
                                     start=True, stop=True)

            # ---- constants -------------------------------------------------
            ident = const.tile([128, 128], BF16)
            make_identity(nc, ident)

            # maskT01[k, q] = 1 where k <= q (valid causal in [k,q] layout)
            maskT01 = const.tile([128, 128], BF16)
            nc.gpsimd.memset(maskT01, 1.0)
            nc.gpsimd.affine_select(
                out=maskT01,
                in_=maskT01,
                compare_op=mybir.AluOpType.is_ge,
                fill=0.0,
                base=0,
                pattern=[[1, 128]],       # expr = -p + f >= 0 -> keep
                channel_multiplier=-1,
            )

            cosf_s = const.tile([128, S], BF16)
            sinf_s = const.tile([128, S], BF16)

            # ---- resident weights (wo loaded later, after proj b0) --------
            wq_s = weights.tile([128, HC, HQ * D], BF16)
            wkv_s = weights.tile([128, HC, 2 * D], BF16)
            for i in range(8):
                sl = slice(i * 4, (i + 1) * 4)
                nc.scalar.dma_start(out=wq_s[:, sl, :], in_=wq_r[:, sl, :])
            wo_s = weights.tile([128, HQ, H], BF16)

            # ---- per-batch activations ------------------------------------
            qT = [acts.tile([128, HQ, S], BF16, tag=f"qT{b}", name=f"qT{b}")
                  for b in range(B)]
            kT = [acts.tile([128, S], BF16, tag=f"kT{b}", name=f"kT{b}")
                  for b in range(B)]
            vnat = [acts.tile([128, KC, VW], BF16, tag=f"vn{b}", name=f"vn{b}")
                    for b in range(B)]

            # ================= Phase 1: projections + rope =================
            with tc.tile_pool(name="pproj", bufs=1, space="PSUM") as psp:
                for b in range(B):
                    tok0 = b * S
                    for t in range(2):  # two 512-token chunks per batch
                        psq = [psp.tile([128, 512], F32, tag="pj", bufs=7,
                                        name=f"psq{b}{t}{d}") for d in range(HQ)]
                        psk = psp.tile([128, 512], F32, tag="pj", bufs=7)
                        psv = psp.tile([128, 512], F32, tag="pj", bufs=7)
                        for hc in range(HC):
                            xp = stream.tile([128, 512], BF16, tag="xp")
                            nc.sync.dma_start(
                                out=xp,
                                in_=xT[hc * 128:(hc + 1) * 128,
                                       tok0 + t * 512: tok0 + (t + 1) * 512],
                            )
                            if b == 0 and t == 0:
                                # wkv/cos/sin ride the sync xp stream just
                                # ahead of their first use
                                if hc % 4 == 0 and hc < 32 - 3:
                                    g = hc // 4
                                    wsl = slice(g * 4, (g + 1) * 4)
                                    nc.sync.dma_start(out=wkv_s[:, wsl, :],
                                                      in_=wkv_r[:, wsl, :])
                                elif hc == 9:
                                    nc.sync.dma_start(out=cosf_s, in_=cosf)
                                elif hc == 13:
                                    nc.sync.dma_start(out=sinf_s, in_=sinf)
                            first, last = hc == 0, hc == HC - 1
                            for d in range(HQ):
                                nc.tensor.matmul(
                                    psq[d],
                                    wq_s[:, hc, d * 128:(d + 1) * 128],
                                    xp,
                                    start=first, stop=last,
                                )
                            nc.tensor.matmul(psk, wkv_s[:, hc, 0:128], xp,
                                             start=first, stop=last)
                            nc.tensor.matmul(psv, wkv_s[:, hc, 128:256], xp,
                                             start=first, stop=last)
                        ts_ = slice(t * 512, (t + 1) * 512)
                        for d in range(HQ):
                            if d % 2 == 0:
                                nc.scalar.copy(qT[b][:, d, ts_], psq[d])
                            else:
                                nc.vector.tensor_copy(qT[b][:, d, ts_], psq[d])
                        nc.vector.tensor_copy(kT[b][:, ts_], psk)
                        # v: transpose to natural [tok, d] via PE
                        vT_sb = work.tile([128, 512], BF16, tag="vT", bufs=2)
                        nc.scalar.copy(vT_sb, psv)
                        ptr = psp.tile([128, 512], BF16, tag="trv", bufs=1,
                                       padded_shape=[128, 1024])
                        for i in range(4):
                            nc.tensor.transpose(
                                ptr[:, i * 128:(i + 1) * 128],
                                vT_sb[:, i * 128:(i + 1) * 128],
                                ident,
                            )
                        nc.vector.tensor_copy(
                            vnat[b][:, t * 4:(t + 1) * 4, 0:128],
                            ptr.rearrange("p (n f) -> p n f", n=4),
                        )
                    # ones column for the softmax denominator; zero pad
                    nc.gpsimd.memset(vnat[b][:, :, 128:129], 1.0)
                    nc.gpsimd.memset(vnat[b][:, :, 129:VW], 0.0)

                    if b == 0:
                        # wo loads ride behind the proj-b0 xp stream; needed
                        # only at the first fused out-proj, ~170us later.
                        for i in range(4):
                            nc.sync.dma_start(out=wo_s[:, i, :],
                                              in_=wo_r[:, i, :])

                    # rope on DVE (overlaps the next batch's projections)
                    def rope(dst):  # [128, S] bf16, in place
                        scr = work.tile([128, S], BF16, tag="scr", bufs=2)
                        nc.gpsimd.dma_start(out=scr[0:64, :], in_=dst[64:128, :])
                        nc.gpsimd.dma_start(out=scr[64:128, :], in_=dst[0:64, :])
                        nc.vector.tensor_mul(dst, dst, cosf_s)
                        nc.vector.tensor_mul(scr, scr, sinf_s)
                        nc.vector.tensor_add(dst, dst, scr)

                    for hh in range(HQ):
                        rope(qT[b][:, hh, :])
                    rope(kT[b])

            # ===== Phase 2: fused attention + output projection ============
            with tc.tile_pool(name="pattn", bufs=1, space="PSUM") as psa:
                aTs = {}

                def outproj(b, tb):
                    tok0 = b * S
                    aT = aTs.pop((b, tb))
                    for pair in range(4):
                        ev = work.tile([128, 1024], BF16, tag="ev", bufs=3)
                        for half in range(2):
                            ncol = pair * 2 + half
                            po = psa.tile([128, 512], F32, tag="op",
                                          bufs=2, name="psop")
                            for d in range(HQ):
                                nc.tensor.matmul(
                                    po,
                                    aT[:, d, :],
                                    wo_s[:, d,
                                         ncol * 512:(ncol + 1) * 512],
                                    start=(d == 0), stop=(d == HQ - 1),
                                )
                            dst = ev[:, half * 512:(half + 1) * 512]
                            if half == 0:
                                nc.scalar.copy(dst, po)
                            else:
                                nc.vector.tensor_copy(dst, po)
                        nc.sync.dma_start(
                            out=out[tok0 + tb * 128: tok0 + (tb + 1) * 128,
                                    pair * 1024:(pair + 1) * 1024],
                            in_=ev,
                        )

                for b in range(B):
                    PT = [work.tile([128, KC, S], BF16, tag=f"pt{hh}", bufs=1,
                                    name=f"PT{hh}") for hh in range(HQ)]
                    for kc in range(KC):
                        qlo = kc * 128
                        # QK^T transposed for all 4 heads at this k-chunk
                        spans = ([(qlo, 512), (512, S)] if qlo < 512
                                 else [(qlo, S)])
                        for hh in range(HQ):
                            for (a, e) in spans:
                                ps = psa.tile([128, 512], F32, tag="qk",
                                              bufs=3, name="psqk")
                                nc.tensor.matmul(
                                    ps[:, :e - a],
                                    kT[b][:, qlo:qlo + 128],
                                    qT[b][:, hh, a:e],
                                    start=True, stop=True,
                                )
                                nc.scalar.activation(
                                    PT[hh][:, kc, a:e],
                                    ps[:, :e - a],
                                    mybir.ActivationFunctionType.Exp,
                                    scale=SCALE,
                                )
                            nc.vector.tensor_mul(
                                PT[hh][:, kc, qlo:qlo + 128],
                                PT[hh][:, kc, qlo:qlo + 128],
                                maskT01,
                            )
                        # out-proj for the previous q-block rides here so
                        # the PE has work while ScalarE exps this k-chunk
                        if kc > 0:
                            outproj(b, kc - 1)
                        elif b > 0:
                            outproj(b - 1, KC - 1)
                        # PV for q-block qb=kc, all heads -> attnT block
                        qb = kc
                        aT = work.tile([128, HQ, 128], BF16, tag="aT", bufs=2)
                        aTs[(b, kc)] = aT
                        for hh in range(HQ):
                            pv = psa.tile([128, VW], F32, tag="pv", bufs=2,
                                          padded_shape=[128, 512], name="pspv")
                            for kc2 in range(qb + 1):
                                nc.tensor.matmul(
                                    pv,
                                    PT[hh][:, kc2, qb * 128:(qb + 1) * 128],
                                    vnat[b][:, kc2, :],
                                    start=(kc2 == 0), stop=(kc2 == qb),
                                )
                            rec = stats.tile([128, 1], F32, tag="st")
                            nc.vector.reciprocal(rec, pv[:, 128:129])
                            an = work.tile([128, 128], BF16, tag="an", bufs=2)
                            nc.vector.tensor_scalar_mul(an, pv[:, 0:128], rec)
                            ptr = psa.tile([128, 128], BF16, tag="tr2", bufs=1,
                                           padded_shape=[128, 1024], name="pstr")
                            nc.tensor.transpose(ptr, an, ident)
                            nc.vector.tensor_copy(aT[:, hh, :], ptr)
                outproj(B - 1, KC - 1)

    nc.compile()
    return nc


_NC = None


def _get_nc():
    global _NC
    if _NC is None:
        _NC = build_program()
    return _NC


def make_in_maps(x, wq, wk, wv, wo, freqs_cos, freqs_sin):
    bf = ml_dtypes.bfloat16
    x = np.asarray(x, np.float32)
    xT = np.ascontiguousarray(x.reshape(NT, H).T.astype(bf))
    cosT = np.asarray(freqs_cos, np.float32).T
    sinT = np.asarray(freqs_sin, np.float32).T
    cosf = np.ascontiguousarray(np.concatenate([cosT, cosT], 0).astype(bf))
    sinf = np.ascontiguousarray(np.concatenate([-sinT, sinT], 0).astype(bf))
    wq = np.asarray(wq, np.float32).astype(bf)
    wk = np.asarray(wk, np.float32).astype(bf)
    wv = np.asarray(wv, np.float32).astype(bf)
    wo = np.asarray(wo, np.float32).astype(bf)

    def pmajor(w):  # [H_in, F] -> [128, (H_in/128)*F] partition-major
        hin, f = w.shape
        return np.ascontiguousarray(
            w.reshape(hin // 128, 128, f).transpose(1, 0, 2).reshape(128, -1))

    in_maps = []
    for c in range(8):
        in_maps.append({
            "xT": xT,
            "wq": pmajor(np.ascontiguousarray(wq[:, c * 512:(c + 1) * 512])),
            "wkv": pmajor(np.ascontiguousarray(
                np.concatenate([wk[:, c * 128:(c + 1) * 128],
                                wv[:, c * 128:(c + 1) * 128]], axis=1))),
            "wo": pmajor(np.ascontiguousarray(wo[c * 512:(c + 1) * 512, :])),
            "cosf": cosf,
            "sinf": sinf,
        })
    return in_maps


def kernel(x, wq, wk, wv, wo, freqs_cos, freqs_sin, start_pos=0, **_):
    nc = _get_nc()
    in_maps = make_in_maps(x, wq, wk, wv, wo, freqs_cos, freqs_sin)
    res = run_bass_kernel_spmd(nc, in_maps, list(range(8)))
    acc = res.results[0]["out"].astype(np.float32)
    for c in range(1, 8):
        acc = acc + res.results[c]["out"].astype(np.float32)
    return acc.reshape(B, S, H)
